# revision 31
# baseline (speedup 1.0000x reference)
"""Trainium2 Bass kernel for nn_EquShiftQ2DF (dense_cnn).

Data-parallel over 8 NeuronCores: each core processes 16 of the 128 samples;
all weights are replicated (host pre-arranges each weight into the exact SBUF
tile layout so every DMA is a contiguous 2D copy).

Per-core pipeline (S=16 samples):
  es branch   : obs [16384,16] -> FC(16384->1024) -> FC(1024->512), FC
                orientation (samples as matmul M via pre-transposed obs).
  ih branch   : conv s2 x2 (as 9-offset shifted matmuls) + FC(4608->512).
  enc convs   : conv1 (host im2col, K=9), conv2/conv3 as 9-offset shifted
                matmuls with K=ci chunks accumulated in PSUM, 2x2 maxpool on
                DVE, pc conv (stride 2).
  hypernet    : df FC -> per-sample filter coeffs; dynamic-filter contraction
                via a fixed-basis matmul (t = basis^T @ H) plus per-sample
                [128x8]x[128x8] matmuls; layout shuffles via one DRAM bounce
                and PE transposes.

Per-stage dtype is configurable: 'bf16' | 'f32r' | 'f32'.  float32r runs the
PE at full rate for moving dims >= 256 with ~1e-4 relative error; activations
for f32r stages are stored as fp32 and bitcast at the matmul.
"""
import numpy as np
import ml_dtypes
from contextlib import ExitStack

import concourse.bass as bass
import concourse.tile as tile
from concourse import bacc, mybir
from concourse.bass_utils import run_bass_kernel_spmd
from concourse.masks import make_identity

NCORES = 8
B = 128
S = B // NCORES  # samples per core

DEFAULT_CFG = {"conv": "bf16", "fc": "bf16", "pc": "bf16"}

_DT_DRAM = {"bf16": mybir.dt.bfloat16, "f32r": mybir.dt.float32r, "f32": mybir.dt.float32}
_DT_ACT = {"bf16": mybir.dt.bfloat16, "f32r": mybir.dt.float32r, "f32": mybir.dt.float32}
_DT_NP = {"bf16": ml_dtypes.bfloat16, "f32r": np.float32, "f32": np.float32}

F32 = mybir.dt.float32


def _mm(ap, key, cfg):
    """Cast an activation AP to the stage's matmul dtype."""
    if cfg[key] == "f32r" and ap.dtype == F32:
        return ap.bitcast(mybir.dt.float32r)
    return ap


def build(cfg):
    nc = bacc.Bacc("TRN2", target_bir_lowering=False, debug=False, num_devices=NCORES)
    cdt = _DT_DRAM[cfg["conv"]]
    fdt = _DT_DRAM[cfg["fc"]]
    pdt = _DT_DRAM[cfg["pc"]]
    cat = _DT_ACT[cfg["conv"]]
    fat = _DT_ACT[cfg["fc"]]
    pat = _DT_ACT[cfg["pc"]]

    D = nc.dram_tensor
    # per-core activations
    # obs k-slice for ALL samples, pre-arranged to [k-part, kc, sample]
    obsT_d = D("obsT", [128, 16, B], fdt, kind="ExternalInput")
    im1_d = D("im1", [9, S, 484], cdt, kind="ExternalInput")
    ihim_d = D("ihim", [9, S, 144], cdt, kind="ExternalInput")
    # replicated weights (host pre-arranged to tile layouts)
    w1r_d = D("w1r", [9, 256], cdt, kind="ExternalInput")
    w2t_d = D("w2t", [128, 2, 9, 512], cdt, kind="ExternalInput")
    w3t_d = D("w3t", [8, 128, 4, 9, 128], cdt, kind="ExternalInput")
    pcwt_d = D("pcwt", [128, 8, 9, 64], pdt, kind="ExternalInput")
    ihw1r_d = D("ihw1r", [9, 64], cdt, kind="ExternalInput")
    ihw2t_d = D("ihw2t", [64, 9, 128], cdt, kind="ExternalInput")
    # ih_fc_w^T k-slice for this core: rows [576*core, 576*(core+1)), padded
    # to 5 chunks of 128 (chunk 4 holds only 64 valid rows)
    ihfcwt_d = D("ihfcwt", [128, 5, 512], fdt, kind="ExternalInput")
    iha2a_in_d = D("iha2a_in", [NCORES, 576, S], fdt)
    iha2a_out_d = D("iha2a_out", [NCORES, 576, S], fdt)
    ihrs_in_d = D("ihrs_in", [B, 512], mybir.dt.bfloat16)
    ihrs_out_d = D("ihrs_out", [S, 512], mybir.dt.bfloat16)
    # es_w1^T k-slice for this core: rows [core*2048, (core+1)*2048)
    esw1t_d = D("esw1t", [2048, 1024], fdt, kind="ExternalInput")
    # ReduceScatter bounce buffers for the sharded es1 FC
    rs_in_d = D("rs_in", [B, 1024], mybir.dt.bfloat16)
    rs_out_d = D("rs_out", [S, 1024], mybir.dt.bfloat16)
    esw2t_d = D("esw2t", [1024, 512], fdt, kind="ExternalInput")
    # df_w^T k-slice for this core: rows [core*128, (core+1)*128)
    dfwt_d = D("dfwt", [128, 2056], fdt, kind="ExternalInput")
    # df collective bounce buffers
    dfa2a_in_d = D("dfa2a_in", [NCORES, 128, S], fdt)
    dfa2a_out_d = D("dfa2a_out", [NCORES, 128, S], fdt)
    dfrs_in_d = D("dfrs_in", [B, 2056], mybir.dt.bfloat16)
    dfrs_out_d = D("dfrs_out", [S, 2056], mybir.dt.bfloat16)
    bprime_d = D("bprime", [72, 256], F32, kind="ExternalInput")
    c2wt_d = D("c2wt", [8, 8, 16], F32, kind="ExternalInput")
    b1_d = D("b1", [256], F32, kind="ExternalInput")
    b2_d = D("b2", [512], F32, kind="ExternalInput")
    b3_d = D("b3", [1024], F32, kind="ExternalInput")
    pcb_d = D("pcb", [64], F32, kind="ExternalInput")
    ihb1_d = D("ihb1", [64], F32, kind="ExternalInput")
    ihb2_d = D("ihb2", [128], F32, kind="ExternalInput")
    esb1_d = D("esb1", [1024], F32, kind="ExternalInput")
    esb2_d = D("esb2", [512], F32, kind="ExternalInput")
    ihfcb_d = D("ihfcb", [512], F32, kind="ExternalInput")
    dfb_d = D("dfb", [2056], F32, kind="ExternalInput")
    c2b_d = D("c2b", [16], F32, kind="ExternalInput")
    out_d = D("out", [S, 2, 8], F32, kind="ExternalOutput")
    scr_t = D("scr_t", [256, 8 * S], F32)   # [(c,d), (g,s)]

    RELU = mybir.ActivationFunctionType.Relu

    with tile.TileContext(nc) as tc, ExitStack() as ctx:
        wts = ctx.enter_context(tc.tile_pool(name="wts", bufs=1))
        stream = ctx.enter_context(tc.tile_pool(name="stream", bufs=3))
        acts = ctx.enter_context(tc.tile_pool(name="acts", bufs=1))
        work = ctx.enter_context(tc.tile_pool(name="work", bufs=2))
        ps = ctx.enter_context(tc.tile_pool(name="ps", bufs=1, space="PSUM"))

        dma = nc.sync.dma_start
        # round-robin DMA issue across the three DMA-capable queues so
        # descriptor generation (~0.6us per call) doesn't serialize
        _qs = [nc.sync.dma_start, nc.scalar.dma_start, nc.gpsimd.dma_start]
        _qi = [0]

        def dma3(out, in_):
            _qs[_qi[0] % 3](out=out, in_=in_)
            _qi[0] += 1

        def dma2(out, in_):
            _qs[_qi[0] % 2](out=out, in_=in_)
            _qi[0] += 1

        # ---- phase 0: constants / weights (conv1/conv2-critical first) ----
        ident = wts.tile([16, 16], F32)
        make_identity(nc, ident)
        w1r_t = wts.tile([9, 256], cdt)
        dma(out=w1r_t, in_=w1r_d.ap())
        b1t = wts.tile([128, 2], F32)
        dma3(out=b1t, in_=b1_d.ap().rearrange("(c p) -> p c", p=128))
        ihw1r_t = wts.tile([9, 64], cdt)
        dma3(out=ihw1r_t, in_=ihw1r_d.ap())
        ihb1t = wts.tile([64, 1], F32)
        dma3(out=ihb1t, in_=ihb1_d.ap().rearrange("(p one) -> p one", one=1))
        w2t_t = wts.tile([128, 2, 9, 512], cdt)
        dma3(out=w2t_t, in_=w2t_d.ap())
        b2t = wts.tile([128, 4], F32)
        dma3(out=b2t, in_=b2_d.ap().rearrange("(c p) -> p c", p=128))
        ihw2t_t = wts.tile([64, 9, 128], cdt)
        dma3(out=ihw2t_t, in_=ihw2t_d.ap())
        ihb2t = wts.tile([128, 1], F32)
        dma3(out=ihb2t, in_=ihb2_d.ap().rearrange("(p one) -> p one", one=1))
        obsT_t = wts.tile([128, 16, B], fdt)
        dma3(out=obsT_t, in_=obsT_d.ap())
        bprime_t = wts.tile([72, 256], F32)
        dma3(out=bprime_t, in_=bprime_d.ap())
        c2wt_t = wts.tile([8, 8, 16], F32)
        dma3(out=c2wt_t, in_=c2wt_d.ap())

        # per-partition conv biases
        b3t = wts.tile([128, 8], F32)
        dma3(out=b3t, in_=b3_d.ap().rearrange("(c p) -> p c", p=128))
        pcbt = wts.tile([64, 1], F32)
        dma3(out=pcbt, in_=pcb_d.ap().rearrange("(p one) -> p one", one=1))
        # FC biases broadcast across the S partitions
        esb1bc = wts.tile([S, 1024], F32)
        dma3(out=esb1bc, in_=bass.AP(esb1_d, 0, [[0, S], [1, 1024]]))
        esb2bc = wts.tile([S, 512], F32)
        dma3(out=esb2bc, in_=bass.AP(esb2_d, 0, [[0, S], [1, 512]]))
        ihfcbc = wts.tile([S, 512], F32)
        dma3(out=ihfcbc, in_=bass.AP(ihfcb_d, 0, [[0, S], [1, 512]]))
        c2bbc = wts.tile([S, 16], F32)
        dma3(out=c2bbc, in_=bass.AP(c2b_d, 0, [[0, S], [1, 16]]))

        # persistent activations
        pooled = acts.tile([128, 4, S, 100], cat)
        pad1 = acts.tile([64, S, 14, 14], cat)
        nc.vector.memset(pad1.bitcast(F32) if pad1.dtype == mybir.dt.float32r else pad1, 0.0)
        ih2act = acts.tile([128, 36, S], fat)
        es1sb = acts.tile([S, 1024], F32)
        es1T = acts.tile([128, 8, S], fat)
        es2T = acts.tile([128, 4, S], fat)
        ihT = acts.tile([128, 4, S], fat)
        wb_sb = acts.tile([S, 2056], F32)

        es1_ps = []
        for h in range(2):
            es1_ps.append(ps.tile([B, 512], F32, tag=f"acc{h}", bufs=1, name=f"es1ps{h}"))

        # ---------- drip-feed unit queue: DMA-heavy FC work interleaved ----------
        from collections import deque
        units = deque()

        def drip(n):
            for _ in range(min(n, len(units))):
                units.popleft()()

        # es1 FC contraction-sharded across cores: this core contracts its
        # 2048-row k-slice against ALL samples, then an 8-way ReduceScatter
        # sums partials and hands each core its own 16 samples.
        def es1_unit(kc):
            def f():
                rw = stream.tile([128, 1024], fdt, tag="esw1", bufs=2, name="esw1t")
                dma(out=rw, in_=esw1t_d.ap()[kc * 128:(kc + 1) * 128, :])
                for h in range(2):
                    nc.tensor.matmul(es1_ps[h], _mm(obsT_t[:, kc, :], "fc", cfg),
                                     rw[:, h * 512:(h + 1) * 512],
                                     start=(kc == 0), stop=(kc == 15),
                                     skip_group_check=True)
            return f

        for kc in range(16):
            units.append(es1_unit(kc))

        box = {}

        def es1_rs_send():
            rs_sb = work.tile([B, 1024], mybir.dt.bfloat16, tag="rssb", bufs=1,
                              name="rs_sb")
            for h in range(2):
                nc.vector.tensor_copy(rs_sb[:, h * 512:(h + 1) * 512], es1_ps[h])
            dma(out=rs_in_d.ap(), in_=rs_sb)

        def es1_cc():
            nc.gpsimd.collective_compute(
                "ReduceScatter", mybir.AluOpType.add,
                replica_groups=[list(range(NCORES))],
                ins=[rs_in_d.ap()], outs=[rs_out_d.ap()])

        units.append(es1_rs_send)
        units.append(es1_cc)

        def es1_fin():
            pre = work.tile([S, 1024], mybir.dt.bfloat16, tag="es1pre", bufs=1,
                            name="es1pre")
            dma(out=pre, in_=rs_out_d.ap())
            nc.vector.tensor_add(es1sb, pre, esb1bc)
            nc.vector.tensor_relu(es1sb, es1sb)

        def transpose_unit(src_fn, dst_fn):
            def f():
                tp = ps.tile([128, S], F32, tag="fc", bufs=2, name="tpos")
                nc.tensor.transpose(tp, src_fn(), ident)
                nc.vector.tensor_copy(dst_fn(), tp)
            return f

        # dynamic-filter L operand + bias transposes (dripped once wb is ready)
        L_sb = acts.tile([128, 2, 8, S], F32)
        dbias_bb = acts.tile([8, 8, S], F32)

        def L_unit(f):
            def g():
                for ch in range(2):
                    tp = ps.tile([128, S], F32, tag="fc", bufs=2, name="tpos")
                    nc.tensor.transpose(
                        tp, wb_sb[:, f * 256 + ch * 128: f * 256 + (ch + 1) * 128],
                        ident)
                    nc.vector.tensor_copy(L_sb[:, ch, f, :], tp)
            return g

        def dbias_unit():
            dbp = ps.tile([8, S], F32, tag="fc", bufs=2, name="dbp")
            nc.tensor.transpose(dbp, wb_sb[:, 2048:2056], ident)
            for d in range(8):
                nc.vector.tensor_copy(dbias_bb[:, d, :], dbp)

        def es2_unit(c):
            def f():
                if c == 0:
                    box["p_es2"] = ps.tile([S, 512], F32, tag="fc", bufs=2,
                                           name="pes2")
                rw = stream.tile([128, 512], fdt, tag="esw2", name="esw2t")
                dma(out=rw, in_=esw2t_d.ap()[c * 128:(c + 1) * 128, :])
                nc.tensor.matmul(box["p_es2"], _mm(es1T[:, c, :], "fc", cfg), rw,
                                 start=(c == 0), stop=(c == 7),
                                 skip_group_check=True)
            return f

        def es2_fin():
            es2sb = work.tile([S, 512], F32, tag="fcout")
            nc.vector.tensor_add(es2sb, box["p_es2"], esb2bc)
            nc.vector.tensor_relu(es2sb, es2sb)
            box["es2sb"] = es2sb

        # ihfc contraction-sharded: this core takes k-rows [576c, 576(c+1))
        # (4.5 chunks of 128); AllToAll redistributes, ReduceScatter sums.
        def iha2a_send():
            ihk = ih2act
            for c in range(NCORES):
                f0 = 576 * c
                sp0, ci0 = f0 // 128, f0 % 128
                base = c * 576 * S
                if ci0 == 0:
                    dma(out=bass.AP(iha2a_in_d, base,
                                    [[S, 128], [128 * S, 4], [1, S]]),
                        in_=ihk[:, sp0:sp0 + 4, :])
                    dma(out=bass.AP(iha2a_in_d, base + 512 * S, [[S, 64], [1, S]]),
                        in_=ihk[0:64, sp0 + 4, :])
                else:
                    dma(out=bass.AP(iha2a_in_d, base, [[S, 64], [1, S]]),
                        in_=ihk[64:128, sp0, :])
                    dma(out=bass.AP(iha2a_in_d, base + 64 * S,
                                    [[S, 128], [128 * S, 4], [1, S]]),
                        in_=ihk[:, sp0 + 1:sp0 + 5, :])

        def iha2a_cc():
            nc.gpsimd.collective_compute(
                "AllToAll", mybir.AluOpType.bypass,
                replica_groups=[list(range(NCORES))],
                ins=[iha2a_in_d.ap()], outs=[iha2a_out_d.ap()])

        def ihfc_mm():
            feat = work.tile([128, 5, NCORES, S], fat, tag="ihfeat", bufs=1,
                             name="ihfeat")
            for d in range(NCORES):
                dma(out=feat[:, 0:4, d, :],
                    in_=bass.AP(iha2a_out_d, d * 576 * S,
                                [[S, 128], [128 * S, 4], [1, S]]))
                dma(out=feat[0:64, 4, d, :],
                    in_=bass.AP(iha2a_out_d, d * 576 * S + 512 * S,
                                [[S, 64], [1, S]]))
            rw = stream.tile([128, 5, 512], fdt, tag="ihfcw", bufs=1, name="ihfcwt")
            dma(out=rw, in_=ihfcwt_d.ap())
            ihp = ps.tile([B, 512], F32, tag="fc", bufs=2, name="pihfc")
            for j in range(4):
                nc.tensor.matmul(ihp, _mm(feat[:, j, :, :], "fc", cfg), rw[:, j, :],
                                 start=(j == 0), stop=False, skip_group_check=True)
            nc.tensor.matmul(ihp, _mm(feat[0:64, 4, :, :], "fc", cfg),
                             rw[0:64, 4, :], start=False, stop=True,
                             skip_group_check=True)
            ihrs_sb = work.tile([B, 512], mybir.dt.bfloat16, tag="ihrs", bufs=1,
                                name="ihrs_sb")
            nc.vector.tensor_copy(ihrs_sb, ihp)
            dma(out=ihrs_in_d.ap(), in_=ihrs_sb)

        def ihrs_cc():
            nc.gpsimd.collective_compute(
                "ReduceScatter", mybir.AluOpType.add,
                replica_groups=[list(range(NCORES))],
                ins=[ihrs_in_d.ap()], outs=[ihrs_out_d.ap()])

        def ihfc_fin():
            pre = work.tile([S, 512], mybir.dt.bfloat16, tag="ihpre", bufs=1,
                            name="ihpre")
            dma(out=pre, in_=ihrs_out_d.ap())
            ihsb = work.tile([S, 512], F32, tag="fcout")
            nc.vector.tensor_add(ihsb, pre, ihfcbc)
            nc.vector.tensor_relu(ihsb, ihsb)
            box["ihsb"] = ihsb

        nsl = [(0, 512), (512, 512), (1024, 512), (1536, 512), (2048, 8)]

        # df FC contraction-sharded: AllToAll redistributes the per-sample
        # feature chunks so this core holds its 128-row k-slice for ALL
        # samples; one local matmul + ReduceScatter yields wb for own samples.
        def dfa2a_send():
            for c in range(NCORES):
                src = es2T[:, c, :] if c < 4 else ihT[:, c - 4, :]
                dma(out=dfa2a_in_d.ap()[c], in_=src)

        def dfa2a_cc():
            nc.gpsimd.collective_compute(
                "AllToAll", mybir.AluOpType.bypass,
                replica_groups=[list(range(NCORES))],
                ins=[dfa2a_in_d.ap()], outs=[dfa2a_out_d.ap()])

        def df_mm():
            dfeat = work.tile([128, NCORES, S], fat, tag="dfeat", bufs=1,
                              name="dfeat")
            dma(out=dfeat, in_=dfa2a_out_d.ap().rearrange("d k s -> k d s"))
            rw = stream.tile([128, 2056], fdt, tag="dfw", bufs=1, name="dfwt_t")
            dma(out=rw, in_=dfwt_d.ap())
            dfrs_sb = work.tile([B, 2056], mybir.dt.bfloat16, tag="dfrs", bufs=1,
                                name="dfrs_sb")
            for ni in range(5):
                n0, nsz = nsl[ni]
                pw = ps.tile([B, 512], F32, tag="fc", bufs=2, name="pwdf")
                nc.tensor.matmul(pw[:, 0:nsz], _mm(dfeat, "fc", cfg),
                                 rw[:, n0:n0 + nsz], start=True, stop=True,
                                 skip_group_check=True)
                nc.vector.tensor_copy(dfrs_sb[:, n0:n0 + nsz], pw[:, 0:nsz])
            dma(out=dfrs_in_d.ap(), in_=dfrs_sb)

        def dfrs_cc():
            nc.gpsimd.collective_compute(
                "ReduceScatter", mybir.AluOpType.add,
                replica_groups=[list(range(NCORES))],
                ins=[dfrs_in_d.ap()], outs=[dfrs_out_d.ap()])

        def df_fin():
            pre = work.tile([S, 2056], mybir.dt.bfloat16, tag="dfpre", bufs=1,
                            name="dfpre")
            dma(out=pre, in_=dfrs_out_d.ap())
            dfbs = work.tile([S, 2056], F32, tag="dfbs", bufs=1, name="dfbs_t")
            dma(out=dfbs, in_=bass.AP(dfb_d, 0, [[0, S], [1, 2056]]))
            nc.vector.tensor_add(wb_sb, pre, dfbs)

        def queue_fc_units():
            units.append(es1_fin)
            for c in range(8):
                units.append(transpose_unit(
                    lambda c=c: es1sb[:, c * 128:(c + 1) * 128],
                    lambda c=c: es1T[:, c, :]))
            for c in range(8):
                units.append(es2_unit(c))
            units.append(es2_fin)
            for c in range(4):
                units.append(transpose_unit(
                    lambda c=c: box["es2sb"][:, c * 128:(c + 1) * 128],
                    lambda c=c: es2T[:, c, :]))
            units.append(iha2a_send)
            units.append(iha2a_cc)
            units.append(ihfc_mm)
            units.append(ihrs_cc)
            units.append(ihfc_fin)
            for c in range(4):
                units.append(transpose_unit(
                    lambda c=c: box["ihsb"][:, c * 128:(c + 1) * 128],
                    lambda c=c: ihT[:, c, :]))
            units.append(dfa2a_send)
            units.append(dfa2a_cc)
            units.append(df_mm)
            units.append(dfrs_cc)
            units.append(df_fin)
            for f in range(8):
                units.append(L_unit(f))
            units.append(dbias_unit)

        # ---- phase 1: conv1 + conv2 + pool, sample-batched; drip between blocks ----
        SG = 1
        with nc.named_scope("phase1_conv12"):
            groups = [list(range(a, min(a + SG, S))) for a in range(0, S, SG)]
            for ss in groups:
                c1os = []
                for s in ss:
                    im1s = work.tile([9, 484], cdt, tag="im1s")
                    dma(out=im1s, in_=im1_d.ap()[:, s, :])
                    c1o = work.tile([128, 2, 484], cat, tag="c1o", bufs=4,
                                    name=f"c1o_{s}")
                    for c in range(2):
                        p1 = ps.tile([128, 484], F32, tag="mm", bufs=3)
                        nc.tensor.matmul(p1, w1r_t[:, c * 128:(c + 1) * 128],
                                         im1s, start=True, stop=True)
                        nc.scalar.activation(c1o[:, c, :], p1, RELU,
                                             bias=b1t[:, c:c + 1])
                    c1os.append(c1o)
                for oc in range(4):
                    p2s = []
                    for k, s in enumerate(ss):
                        p2s.append(ps.tile([128, 400], F32, tag="mm", bufs=3,
                                           name=f"p2_{s}"))
                    for c in range(2):
                        for ky in range(3):
                            for kx in range(3):
                                w = w2t_t[:, c, ky * 3 + kx, oc * 128:(oc + 1) * 128]
                                for k in range(len(ss)):
                                    c1v = c1os[k].rearrange(
                                        "p c (h w) -> p c h w", h=22)
                                    nc.tensor.matmul(
                                        p2s[k], w,
                                        _mm(c1v[:, c, ky:ky + 20, kx:kx + 20], "conv", cfg),
                                        start=(c == 0 and ky == 0 and kx == 0),
                                        stop=(c == 1 and ky == 2 and kx == 2),
                                        skip_group_check=True)
                    for k, s in enumerate(ss):
                        c2o = work.tile([128, 20, 20], cat, tag="c2o")
                        nc.scalar.activation(
                            c2o, p2s[k].rearrange("p (h w) -> p h w", h=20),
                            RELU, bias=b2t[:, oc:oc + 1])
                        tmp = work.tile([128, 10, 20], cat, tag="pooltmp", bufs=1)
                        nc.vector.tensor_max(tmp, c2o[:, 0:20:2, :], c2o[:, 1:20:2, :])
                        nc.vector.tensor_max(
                            pooled[:, oc, s, :].rearrange("p (h w) -> p h w", h=10),
                            tmp[:, :, 0:20:2], tmp[:, :, 1:20:2])
                # ih conv1 for these samples
                for s in ss:
                    ihims = work.tile([9, 144], cdt, tag="ihims")
                    dma(out=ihims, in_=ihim_d.ap()[:, s, :])
                    pi = ps.tile([64, 144], F32, tag="mm", bufs=3)
                    nc.tensor.matmul(pi, ihw1r_t, ihims, start=True, stop=True)
                    nc.scalar.activation(pad1[:, s, 1:13, 1:13],
                                         pi.rearrange("p (h w) -> p h w", h=12),
                                         RELU, bias=ihb1t)
                drip(8)

        # ---- phase 2: ih conv2 ----
        with nc.named_scope("phase2_ih2"):
            for grp in range(2):
                p2i = ps.tile([128, 8, 36], F32, tag="mm", bufs=3)
                sl = slice(grp * 8, (grp + 1) * 8)
                for ky in range(3):
                    for kx in range(3):
                        nc.tensor.matmul(
                            p2i, ihw2t_t[:, ky * 3 + kx, :],
                            _mm(pad1[:, sl, ky:ky + 12:2, kx:kx + 12:2], "conv", cfg),
                            start=(ky == 0 and kx == 0), stop=(ky == 2 and kx == 2))
                nc.scalar.activation(
                    ih2act.rearrange("p k s -> p s k")[:, sl, :], p2i,
                    RELU, bias=ihb2t)
            queue_fc_units()

        # ---- phase 3: conv3, remaining units dripped between ci-chunks ----
        with nc.named_scope("phase3_conv3"):
            pv = pooled.rearrange("p c s (h w) -> p c s h w", h=10)
            ppc = ps.tile([64, 9, S], F32, tag="ppc", bufs=1)

            # pc conv rhs windows have 2-byte runs (stride-2 into an 8x8 map)
            # which cripple the PE's SBUF stream reads; build a contiguous
            # im2col buffer on DVE and run the pc matmuls one oc behind.
            def emit_pc(oc, pcim):
                pcw_c = stream.tile([128, 9, 64], pdt, tag="pcw", bufs=2)
                dma(out=pcw_c, in_=pcwt_d.ap()[:, oc, :, :])
                for kk in range(9):
                    nc.tensor.matmul(ppc, pcw_c[:, kk, :],
                                     _mm(pcim[:, kk, :, :, :], "pc", cfg),
                                     start=(oc == 0 and kk == 0),
                                     stop=(oc == 7 and kk == 8),
                                     skip_group_check=True)

            pcim_prev = None
            for oc in range(8):
                p3 = []
                for grp in range(2):
                    p3.append(ps.tile([128, 8, 64], F32, tag="mm", bufs=3,
                                      name=f"p3g{grp}"))
                for c in range(4):
                    w3c = stream.tile([128, 9, 128], cdt, tag="w3", bufs=2)
                    dma(out=w3c, in_=w3t_d.ap()[oc, :, c, :, :])
                    for ky in range(3):
                        for kx in range(3):
                            for grp in range(2):
                                sl = slice(grp * 8, (grp + 1) * 8)
                                nc.tensor.matmul(
                                    p3[grp], w3c[:, ky * 3 + kx, :],
                                    _mm(pv[:, c, sl, ky:ky + 8, kx:kx + 8], "conv", cfg),
                                    start=(c == 0 and ky == 0 and kx == 0),
                                    stop=(c == 3 and ky == 2 and kx == 2),
                                    skip_group_check=True)
                    drip(3)
                    if c == 1 and pcim_prev is not None:
                        emit_pc(oc - 1, pcim_prev)
                c3a = work.tile([128, S, 64], pat, tag="c3a", bufs=2)
                for grp in range(2):
                    sl = slice(grp * 8, (grp + 1) * 8)
                    nc.scalar.activation(c3a[:, sl, :], p3[grp], RELU,
                                         bias=b3t[:, oc:oc + 1])
                pcim = work.tile([128, 9, 3, 3, S], pat, tag="pcim", bufs=2)
                for kk in range(9):
                    ky, kx = kk // 3, kk % 3
                    src = bass.AP(c3a.tensor, c3a.offset + ky * 8 + kx,
                                  [c3a.ap[0], [16, 3], [2, 3], [64, S]])
                    # on Scalar: the Vector queue carries collective-dependent
                    # copies that would head-block this conv3-paced work
                    nc.scalar.activation(pcim[:, kk, :, :, :], src,
                                         mybir.ActivationFunctionType.Copy)
                pcim_prev = pcim
            emit_pc(7, pcim_prev)

        # ---- phase 4: pc epilogue ----
        with nc.named_scope("phase4_pc"):
            pc_act = work.tile([64, 9, S], F32, tag="pcact", bufs=1)
            nc.scalar.activation(pc_act, ppc, RELU, bias=pcbt)

        # ---- phase 4b: H shuffle + basis stage (overlaps the FC drain) ----
        with nc.named_scope("phase4b_basis"):
            # H'' regroup [(g,e), kk, s] -> [(e,kk), g, s] via sbuf->sbuf DMA
            H_t = work.tile([72, 8, S], F32, tag="Ht", bufs=1)
            for g in range(8):
                dma2(out=H_t[:, g, :], in_=pc_act[g * 8:(g + 1) * 8, :, :])
            # stage 1: t'' [(c,d) 2x128, (g,s) 128]
            tps = ps.tile([128, 2, 8, S], F32, tag="fc", bufs=2)
            for ch in range(2):
                nc.tensor.matmul(tps[:, ch, :, :], bprime_t[:, ch * 128:(ch + 1) * 128],
                                 H_t, start=True, stop=True)
            t_sb = work.tile([128, 2, 8, S], F32, tag="tsb", bufs=1)
            nc.vector.tensor_copy(t_sb, tps)
            for ch in range(2):
                dst = bass.AP(scr_t, ch * 128 * 8 * S, [[8 * S, 128], [S, 8], [1, S]])
                dma2(out=dst, in_=t_sb[:, ch, :, :])
            R_t = work.tile([128, 2, 8, S], F32, tag="Rt", bufs=1)
            for g in range(8):
                ch, gq = g // 4, g % 4
                src = bass.AP(scr_t, g * S, [[8 * 8 * S, 32], [8 * S, 8], [1, S]])
                dma2(out=R_t[gq * 32:(gq + 1) * 32, ch, :, :], in_=src)

        # ---- phase 5: drain remaining units ----
        with nc.named_scope("phase5_fc"):
            drip(len(units))

        # ---- phase 6: dynamic filter tail (all f32) ----
        with nc.named_scope("phase6_tail"):
            # stage 2: out2 [8(f), 8(d), S]
            o2 = ps.tile([8, 8, S], F32, tag="fc", bufs=2)
            for s in range(S):
                for ch in range(2):
                    nc.tensor.matmul(o2[:, :, s], L_sb[:, ch, :, s], R_t[:, ch, :, s],
                                     start=(ch == 0), stop=(ch == 1))
            feat = work.tile([8, 8, S], F32, tag="feat", bufs=1)
            nc.vector.tensor_add(feat, o2, dbias_bb)
            nc.vector.tensor_relu(feat, feat)
            xps = ps.tile([S, 16], F32, tag="fc", bufs=2)
            for d in range(8):
                nc.tensor.matmul(xps, feat[:, d, :], c2wt_t[:, d, :],
                                 start=(d == 0), stop=(d == 7))
            x_sb = work.tile([S, 16], F32, tag="xsb", bufs=1)
            nc.vector.tensor_add(x_sb, xps, c2bbc)
            dma(out=out_d.ap(), in_=x_sb)

    nc.compile()
    return nc


def _prep_inputs(inputs, cfg):
    """Host-side: shard activations, rearrange weights into tile layouts."""
    i = {k: np.asarray(v, dtype=np.float32) for k, v in inputs.items()}
    cnp = _DT_NP[cfg["conv"]]
    fnp = _DT_NP[cfg["fc"]]
    pnp = _DT_NP[cfg["pc"]]

    obs = i["obs_encoding"].reshape(B, 16384)
    image = i["patch"][:, 0]   # [B,24,24]
    inhand = i["patch"][:, 1]  # [B,24,24]

    # conv1 im2col: [B, 9, 484]
    sw = np.lib.stride_tricks.sliding_window_view(image, (3, 3), axis=(1, 2))
    im1 = sw.transpose(0, 3, 4, 1, 2).reshape(B, 9, 484)
    # ih conv1 im2col (stride 2, pad 1): [B, 9, 144]
    ip = np.pad(inhand, ((0, 0), (1, 1), (1, 1)))
    swi = np.lib.stride_tricks.sliding_window_view(ip, (3, 3), axis=(1, 2))[:, ::2, ::2]
    ihim = swi.transpose(0, 3, 4, 1, 2).reshape(B, 9, 144)

    def conv_w_t(w, nchunk):  # [O, I, 3, 3] -> [128, nchunk, 9, O]
        O, I = w.shape[0], w.shape[1]
        return np.ascontiguousarray(
            w.reshape(O, nchunk, 128, 9).transpose(2, 1, 3, 0))

    shared = {
        "w1r": np.ascontiguousarray(i["enc_w1"].reshape(256, 9).T).astype(cnp),
        "w2t": conv_w_t(i["enc_w2"], 2).astype(cnp),
        "w3t": np.ascontiguousarray(
            i["enc_w3"].reshape(8, 128, 4, 128, 9).transpose(0, 3, 2, 4, 1)).astype(cnp),
        "pcwt": conv_w_t(i["pc_w"], 8).astype(pnp),
        "ihw1r": np.ascontiguousarray(i["ih_w1"].reshape(64, 9).T).astype(cnp),
        "ihw2t": np.ascontiguousarray(
            i["ih_w2"].reshape(128, 64, 9).transpose(1, 2, 0)).astype(cnp),

        "esw2t": np.ascontiguousarray(i["es_w2"].T).astype(fnp),
        # basis [c,d,e,kh,kw] -> [(e,kk), (c,d)]
        "bprime": np.ascontiguousarray(
            i["basis"].reshape(32, 8, 8, 9).transpose(2, 3, 0, 1).reshape(72, 256)),
        # c2_w [o, f*8+d] -> [f, d, o]
        "c2wt": np.ascontiguousarray(i["c2_w"].reshape(16, 8, 8).transpose(1, 2, 0)),
        "b1": i["enc_b1"], "b2": i["enc_b2"], "b3": i["enc_b3"],
        "pcb": i["pc_b"], "ihb1": i["ih_b1"], "ihb2": i["ih_b2"],
        "esb1": i["es_b1"], "esb2": i["es_b2"], "ihfcb": i["ih_fc_b"],
        "dfb": i["df_b"], "c2b": i["c2_b"],
    }
    esw1T = i["es_w1"].T  # [16384, 1024]
    dfwT = i["df_w"].T    # [1024, 2056]
    # ih_fc_w^T with feature order f' = sp*128 + ci (ih flatten is ci*36+sp)
    ihwT = np.ascontiguousarray(
        i["ih_fc_w"].reshape(512, 128, 36).transpose(2, 1, 0).reshape(4608, 512))
    in_maps = []
    for c in range(NCORES):
        sl = slice(c * S, (c + 1) * S)
        ksl = slice(c * 2048, (c + 1) * 2048)
        m = dict(shared)
        # obs k-slice for ALL samples, in tile layout [k-part, kc, sample]
        m["obsT"] = np.ascontiguousarray(
            obs[:, ksl].T.reshape(16, 128, B).transpose(1, 0, 2)).astype(fnp)
        m["esw1t"] = np.ascontiguousarray(esw1T[ksl]).astype(fnp)
        m["dfwt"] = np.ascontiguousarray(dfwT[c * 128:(c + 1) * 128]).astype(fnp)
        w5 = np.zeros((640, 512), np.float32)
        w5[:576] = ihwT[576 * c: 576 * (c + 1)]
        m["ihfcwt"] = np.ascontiguousarray(
            w5.reshape(5, 128, 512).transpose(1, 0, 2)).astype(fnp)
        m["im1"] = np.ascontiguousarray(im1[sl].transpose(1, 0, 2)).astype(cnp)
        m["ihim"] = np.ascontiguousarray(ihim[sl].transpose(1, 0, 2)).astype(cnp)
        in_maps.append(m)
    return in_maps


_CACHE = {}


def _get_nc(cfg):
    key = tuple(sorted(cfg.items()))
    if key not in _CACHE:
        _CACHE[key] = build(cfg)
    return _CACHE[key]


def run(inputs, cfg=None, trace=False):
    cfg = cfg or DEFAULT_CFG
    nc = _get_nc(cfg)
    in_maps = _prep_inputs(inputs, cfg)
    res = run_bass_kernel_spmd(nc, in_maps, list(range(NCORES)), trace=trace)
    out = np.concatenate([res.results[c]["out"] for c in range(NCORES)], axis=0)
    return out.astype(np.float32), res


def kernel(**inputs) -> np.ndarray:
    out, _ = run(inputs)
    return out

